# revision 18
# baseline (speedup 1.0000x reference)
"""Trainium2 Bass kernel for nn_AVDFullLinearMix.

Math (folded form, all terms single matmuls over raw inputs):
  x_d_out = x_d + W_ttrans @ x_d^T(spatial) + (W_tdelta @ x_a) * eye3
            + eps_expand(W_vd @ x_v)
  x_a_out = x_a + W_ct @ trace(x_d),        W_ct = W_ttrace @ (I + W_ttrans)
  x_v_out = x_v + W_cd @ eps_contract(x_d), W_cd = W_dv @ (I - W_ttrans)
(the TensDelta term never reaches the eps-contract since eps[i,i,k] = 0,
 and trace/eps-contract commute with the TensTrans spatial transpose up to
 identity/negation, so both weight chains fold on the host.)

Sharding: data-parallel over 8 cores (2048 tokens each); weights replicated.
Device layout is channel-major and chunk-packed: one input tensor
[256, chunk, 13, tok] per core holding (9 x_d planes, 3 x_v planes, 1 x_a)
so each chunk is ONE contiguous in-DMA (sync ring) while outputs stream on
the scalar HWDGE ring. Matmuls run as float32r (fp22 multiply, fp32
accumulate) at full PE rate; both output-channel halves share one PSUM bank
so each spatial plane needs a single DVE combine op.
"""

import numpy as np

import concourse.bass as bass  # noqa: F401  (registers handle types)
import concourse.mybir as mybir
import concourse.tile as tile
from concourse import bacc
from concourse.bass_utils import run_bass_kernel_spmd

NCORES = 8
P = 128          # partitions
C = 256          # channels
B, N = 16, 1024
T = (B * N) // NCORES   # tokens per core = 2048
TCH = 256               # tokens per chunk
NCHUNK = T // TCH       # 8
NPL = 13                # planes per chunk: 9 x_d, 3 x_v, 1 x_a
PL_XV = 9
PL_XA = 12
OUT_SPLIT = 7           # out-DMA part A = planes [0:7), part B = [7:13)

F32 = mybir.dt.float32
R32 = mybir.dt.float32r

# flat spatial index s = i*3 + j
SPERM = [0, 3, 6, 1, 4, 7, 2, 5, 8]    # s -> transposed flat (j*3+i)
DIAG = (0, 4, 8)
# off-diag s=(k,j): x_d_out[...,k,j] += eps[i*,j,k] * (W_vd @ x_v[..,i*])
# VD_MAP: s -> (i*, weight slot) with slot 2 = +W_vd^T, 3 = -W_vd^T
VD_MAP = {1: (2, 3), 2: (1, 2), 3: (2, 2), 5: (0, 3), 6: (1, 3), 7: (0, 2)}
# eps-contract y[:,k] = x_d[:,s1] - x_d[:,s2]
Y_PAIRS = [(5, 7), (6, 2), (1, 3)]
# weight slots (lhsT layout W.T): 0=ttrans 1=tdelta 2=+vd 3=-vd 4=cd 5=ct
NW = 6

_CACHE: dict = {}


def _build():
    nc = bacc.Bacc(None, target_bir_lowering=False)
    xin = nc.dram_tensor("xin", [C, NCHUNK, NPL, TCH], F32, kind="ExternalInput")
    wts = nc.dram_tensor("wts", [NW, C, C], F32, kind="ExternalInput")
    xout = nc.dram_tensor("xout", [C, NCHUNK, NPL, TCH], F32, kind="ExternalOutput")

    # channel-split dram views: c = h*128 + p
    xin_r = xin.rearrange("(h p) n s t -> p h n s t", p=P)
    xout_r = xout.rearrange("(h p) n s t -> p h n s t", p=P)
    wts_r = wts.rearrange("w (kh p) o -> p w kh o", p=P)

    with tile.TileContext(nc) as tc:
        with (
            tc.tile_pool(name="wpool", bufs=1) as wpool,
            tc.tile_pool(name="data", bufs=3) as data,
            tc.tile_pool(name="outp", bufs=3) as outp,
            tc.tile_pool(name="tmp", bufs=2) as tmp,
            tc.tile_pool(name="psum", bufs=8, space="PSUM") as psum,
        ):
            w_sb = wpool.tile([P, NW, 2, C], F32)
            nc.sync.dma_start(w_sb[:].bitcast(R32), wts_r.bitcast(R32))

            def lhsT(w, kh, mh):
                return w_sb[:, w, kh, mh * P:(mh + 1) * P].bitcast(R32)

            x_tiles = {}

            def prefetch(ck, part=None):
                if ck >= NCHUNK:
                    return
                if ck not in x_tiles:
                    x_tiles[ck] = data.tile([P, 2, NPL, TCH], F32, name="x", tag="x")
                xt = x_tiles[ck]
                lo, hi = {
                    None: (0, NPL), 0: (0, OUT_SPLIT), 1: (OUT_SPLIT, NPL)
                }[part]
                nc.sync.dma_start(
                    xt[:, :, lo:hi, :].bitcast(R32),
                    xin_r[:, :, ck, lo:hi].bitcast(R32),
                )

            # 3-chunk in-DMA lookahead keeps the sync HWDGE ring free of
            # head-of-line blocking; in/out parts alternate so reads and
            # writes overlap in the SDMA engines (higher aggregate HBM BW)
            for ck in range(3):
                prefetch(ck)

            for ck in range(NCHUNK):
                x_sb = x_tiles[ck]
                o_sb = outp.tile([P, 2, NPL, TCH], F32, tag="o")

                # eps-contract and trace of raw x_d; fp32r-typed for the matmuls
                y_sb = tmp.tile([P, 2, 3, TCH], F32, tag="y")
                tr_sb = tmp.tile([P, 2, TCH], F32, tag="tr")
                for k, (s1, s2) in enumerate(Y_PAIRS):
                    nc.vector.tensor_sub(
                        y_sb[:, :, k, :].bitcast(R32),
                        x_sb[:, :, s1, :], x_sb[:, :, s2, :],
                    )
                nc.vector.tensor_add(
                    tr_sb[:].bitcast(R32), x_sb[:, :, 0, :], x_sb[:, :, 4, :]
                )
                nc.vector.tensor_add(
                    tr_sb[:].bitcast(R32), tr_sb[:], x_sb[:, :, 8, :]
                )

                def mm_group(ps, terms):
                    """ps[:, mh, :] (+)= sum of W[wslot].T @ rhs(kh) terms, both
                    output-channel halves into one PSUM bank."""
                    for mh in range(2):
                        n = 0
                        total = len(terms) * 2
                        for wslot, rhs in terms:
                            for kh in range(2):
                                n += 1
                                nc.tensor.matmul(
                                    ps[:, mh, :], lhsT(wslot, kh, mh), rhs(kh),
                                    start=(n == 1), stop=(n == total),
                                )

                # x_d planes
                for s in range(9):
                    ps = psum.tile([P, 2, TCH], F32, tag="ps")
                    if s in DIAG:
                        extra = (1, lambda kh: x_sb[:, kh, PL_XA, :].bitcast(R32))
                    else:
                        istar, wslot = VD_MAP[s]
                        extra = (
                            wslot,
                            lambda kh, i=istar: x_sb[:, kh, PL_XV + i, :].bitcast(R32),
                        )
                    mm_group(ps, [
                        (0, lambda kh, sp=SPERM[s]: x_sb[:, kh, sp, :].bitcast(R32)),
                        extra,
                    ])
                    nc.vector.tensor_add(o_sb[:, :, s, :], x_sb[:, :, s, :], ps[:])

                # x_v planes: xv += W_cd @ y
                for k in range(3):
                    ps = psum.tile([P, 2, TCH], F32, tag="ps")
                    mm_group(ps, [
                        (4, lambda kh, kk=k: y_sb[:, kh, kk, :].bitcast(R32)),
                    ])
                    nc.vector.tensor_add(
                        o_sb[:, :, PL_XV + k, :], x_sb[:, :, PL_XV + k, :], ps[:]
                    )

                # x_a plane: xa += W_ct @ tr
                ps = psum.tile([P, 2, TCH], F32, tag="ps")
                mm_group(ps, [
                    (5, lambda kh: tr_sb[:, kh, :].bitcast(R32)),
                ])
                nc.vector.tensor_add(
                    o_sb[:, :, PL_XA, :], x_sb[:, :, PL_XA, :], ps[:]
                )

                nc.sync.dma_start(
                    xout_r[:, :, ck, :OUT_SPLIT], o_sb[:, :, :OUT_SPLIT, :]
                )
                prefetch(ck + 3)
                nc.sync.dma_start(
                    xout_r[:, :, ck, OUT_SPLIT:], o_sb[:, :, OUT_SPLIT:, :]
                )
    nc.compile()
    return nc


def _get_nc():
    if "nc" not in _CACHE:
        _CACHE["nc"] = _build()
    return _CACHE["nc"]


def kernel(x_a, x_v, x_d, W_ttrans, W_ttrace, W_tdelta, W_vd, W_dv, **_ignored):
    x_a = np.asarray(x_a, dtype=np.float32)
    x_v = np.asarray(x_v, dtype=np.float32)
    x_d = np.asarray(x_d, dtype=np.float32)
    W_ttrans = np.asarray(W_ttrans, dtype=np.float32)
    W_ttrace = np.asarray(W_ttrace, dtype=np.float32)
    W_tdelta = np.asarray(W_tdelta, dtype=np.float32)
    W_vd = np.asarray(W_vd, dtype=np.float32)
    W_dv = np.asarray(W_dv, dtype=np.float32)

    eye = np.eye(C, dtype=np.float32)
    W_ct = (W_ttrace @ (eye + W_ttrans)).astype(np.float32)
    W_cd = (W_dv @ (eye - W_ttrans)).astype(np.float32)
    wts = np.ascontiguousarray(
        np.stack([
            W_ttrans.T, W_tdelta.T, W_vd.T, -W_vd.T, W_cd.T, W_ct.T,
        ]).astype(np.float32)
    )

    # host reshard: pack (x_d, x_v, x_a) chunk-plane-major, channel on outer dim
    xin = np.empty((NCORES, C, NCHUNK, NPL, TCH), dtype=np.float32)
    xin[:, :, :, 0:9, :] = (
        x_d.reshape(NCORES, NCHUNK, TCH, C, 9).transpose(0, 3, 1, 4, 2)
    )
    xin[:, :, :, 9:12, :] = (
        x_v.reshape(NCORES, NCHUNK, TCH, C, 3).transpose(0, 3, 1, 4, 2)
    )
    xin[:, :, :, 12, :] = (
        x_a.reshape(NCORES, NCHUNK, TCH, C).transpose(0, 3, 1, 2)
    )

    in_maps = [{"xin": xin[c], "wts": wts} for c in range(NCORES)]

    nc = _get_nc()
    res = run_bass_kernel_spmd(nc, in_maps, core_ids=list(range(NCORES)))

    xout = np.stack([res.results[c]["xout"] for c in range(NCORES)])
    # [core, C, chunk, plane, t] -> [core, chunk, t, C, plane]
    xout = xout.transpose(0, 2, 4, 1, 3)
    x_d_out = np.ascontiguousarray(xout[..., 0:9]).reshape(B, N, C, 3, 3)
    x_v_out = np.ascontiguousarray(xout[..., 9:12]).reshape(B, N, C, 3)
    x_a_out = np.ascontiguousarray(xout[..., 12]).reshape(B, N, C)
    return (x_a_out, x_v_out, x_d_out)


# revision 20
# speedup vs baseline: 1.3041x; 1.3041x over previous
"""Trainium2 Bass kernel for nn_AVDFullLinearMix.

Math (folded form, all terms single matmuls over raw inputs):
  x_d_out = x_d + W_ttrans @ x_d^T(spatial) + (W_tdelta @ x_a) * eye3
            + eps_expand(W_vd @ x_v)
  x_a_out = x_a + W_ct @ trace(x_d),        W_ct = W_ttrace @ (I + W_ttrans)
  x_v_out = x_v + W_cd @ eps_contract(x_d), W_cd = W_dv @ (I - W_ttrans)
(the TensDelta term never reaches the eps-contract since eps[i,i,k] = 0,
 and trace/eps-contract commute with the TensTrans spatial transpose up to
 identity/negation, so both weight chains fold on the host.)

Sharding: data-parallel over 8 cores (2048 tokens each); weights replicated.
Device layout is channel-major and chunk-packed: one input tensor
[256, chunk, 13, tok] per core holding (9 x_d planes, 3 x_v planes, 1 x_a)
so each chunk is ONE contiguous in-DMA (sync ring) while outputs stream on
the scalar HWDGE ring. Matmuls run as float32r (fp22 multiply, fp32
accumulate) at full PE rate; both output-channel halves share one PSUM bank
so each spatial plane needs a single DVE combine op.
"""

import numpy as np

import concourse.bass as bass  # noqa: F401  (registers handle types)
import concourse.mybir as mybir
import concourse.tile as tile
from concourse import bacc
from concourse.bass_utils import run_bass_kernel_spmd

NCORES = 8
P = 128          # partitions
C = 256          # channels
B, N = 16, 1024
T = (B * N) // NCORES   # tokens per core = 2048
TCH = 256               # tokens per chunk
NCHUNK = T // TCH       # 8
NPL = 13                # planes per chunk: 9 x_d, 3 x_v, 1 x_a
PL_XV = 9
PL_XA = 12
OUT_SPLIT = 7           # out-DMA part A = planes [0:7), part B = [7:13)

F32 = mybir.dt.float32
F16 = mybir.dt.float16
R32 = mybir.dt.float32r

# flat spatial index s = i*3 + j
SPERM = [0, 3, 6, 1, 4, 7, 2, 5, 8]    # s -> transposed flat (j*3+i)
DIAG = (0, 4, 8)
# off-diag s=(k,j): x_d_out[...,k,j] += eps[i*,j,k] * (W_vd @ x_v[..,i*])
# VD_MAP: s -> (i*, weight slot) with slot 2 = +W_vd^T, 3 = -W_vd^T
VD_MAP = {1: (2, 3), 2: (1, 2), 3: (2, 2), 5: (0, 3), 6: (1, 3), 7: (0, 2)}
# eps-contract y[:,k] = x_d[:,s1] - x_d[:,s2]
Y_PAIRS = [(5, 7), (6, 2), (1, 3)]
# weight slots (lhsT layout W.T): 0=ttrans 1=tdelta 2=+vd 3=-vd 4=cd 5=ct
NW = 6

_CACHE: dict = {}


def _build():
    nc = bacc.Bacc(None, target_bir_lowering=False)
    xin = nc.dram_tensor("xin", [C, NCHUNK, NPL, TCH], F32, kind="ExternalInput")
    wts = nc.dram_tensor("wts", [NW, C, C], F32, kind="ExternalInput")
    xout = nc.dram_tensor("xout", [C, NCHUNK, NPL, TCH], F16, kind="ExternalOutput")

    # channel-split dram views: c = h*128 + p
    xin_r = xin.rearrange("(h p) n s t -> p h n s t", p=P)
    xout_r = xout.rearrange("(h p) n s t -> p h n s t", p=P)
    wts_r = wts.rearrange("w (kh p) o -> p w kh o", p=P)

    with tile.TileContext(nc) as tc:
        with (
            tc.tile_pool(name="wpool", bufs=1) as wpool,
            tc.tile_pool(name="data", bufs=3) as data,
            tc.tile_pool(name="outp", bufs=3) as outp,
            tc.tile_pool(name="tmp", bufs=2) as tmp,
            tc.tile_pool(name="psum", bufs=8, space="PSUM") as psum,
        ):
            w_sb = wpool.tile([P, NW, 2, C], F32)
            nc.sync.dma_start(w_sb[:].bitcast(R32), wts_r.bitcast(R32))

            def lhsT(w, kh, mh):
                return w_sb[:, w, kh, mh * P:(mh + 1) * P].bitcast(R32)

            x_tiles = {}

            def prefetch(ck, part=None):
                if ck >= NCHUNK:
                    return
                if ck not in x_tiles:
                    x_tiles[ck] = data.tile([P, 2, NPL, TCH], F32, name="x", tag="x")
                xt = x_tiles[ck]
                lo, hi = {
                    None: (0, NPL), 0: (0, OUT_SPLIT), 1: (OUT_SPLIT, NPL)
                }[part]
                nc.sync.dma_start(
                    xt[:, :, lo:hi, :].bitcast(R32),
                    xin_r[:, :, ck, lo:hi].bitcast(R32),
                )

            # 3-chunk in-DMA lookahead keeps the sync HWDGE ring free of
            # head-of-line blocking; in/out parts alternate so reads and
            # writes overlap in the SDMA engines (higher aggregate HBM BW)
            for ck in range(3):
                prefetch(ck)

            for ck in range(NCHUNK):
                x_sb = x_tiles[ck]
                o_sb = outp.tile([P, 2, NPL, TCH], F16, tag="o")

                # eps-contract and trace of raw x_d; fp32r-typed for the matmuls
                y_sb = tmp.tile([P, 2, 3, TCH], F32, tag="y")
                tr_sb = tmp.tile([P, 2, TCH], F32, tag="tr")
                for k, (s1, s2) in enumerate(Y_PAIRS):
                    nc.vector.tensor_sub(
                        y_sb[:, :, k, :].bitcast(R32),
                        x_sb[:, :, s1, :], x_sb[:, :, s2, :],
                    )
                nc.vector.tensor_add(
                    tr_sb[:].bitcast(R32), x_sb[:, :, 0, :], x_sb[:, :, 4, :]
                )
                nc.vector.tensor_add(
                    tr_sb[:].bitcast(R32), tr_sb[:], x_sb[:, :, 8, :]
                )

                def mm_group(ps, terms):
                    """ps[:, mh, :] (+)= sum of W[wslot].T @ rhs(kh) terms, both
                    output-channel halves into one PSUM bank."""
                    for mh in range(2):
                        n = 0
                        total = len(terms) * 2
                        for wslot, rhs in terms:
                            for kh in range(2):
                                n += 1
                                nc.tensor.matmul(
                                    ps[:, mh, :], lhsT(wslot, kh, mh), rhs(kh),
                                    start=(n == 1), stop=(n == total),
                                )

                # x_d planes
                for s in range(9):
                    ps = psum.tile([P, 2, TCH], F32, tag="ps")
                    if s in DIAG:
                        extra = (1, lambda kh: x_sb[:, kh, PL_XA, :].bitcast(R32))
                    else:
                        istar, wslot = VD_MAP[s]
                        extra = (
                            wslot,
                            lambda kh, i=istar: x_sb[:, kh, PL_XV + i, :].bitcast(R32),
                        )
                    mm_group(ps, [
                        (0, lambda kh, sp=SPERM[s]: x_sb[:, kh, sp, :].bitcast(R32)),
                        extra,
                    ])
                    nc.vector.tensor_copy(o_sb[:, :, s, :], ps[:])

                # x_v planes: xv += W_cd @ y
                for k in range(3):
                    ps = psum.tile([P, 2, TCH], F32, tag="ps")
                    mm_group(ps, [
                        (4, lambda kh, kk=k: y_sb[:, kh, kk, :].bitcast(R32)),
                    ])
                    nc.vector.tensor_copy(o_sb[:, :, PL_XV + k, :], ps[:])

                # x_a plane: xa += W_ct @ tr
                ps = psum.tile([P, 2, TCH], F32, tag="ps")
                mm_group(ps, [
                    (5, lambda kh: tr_sb[:, kh, :].bitcast(R32)),
                ])
                nc.vector.tensor_copy(o_sb[:, :, PL_XA, :], ps[:])

                nc.sync.dma_start(
                    xout_r[:, :, ck, :OUT_SPLIT], o_sb[:, :, :OUT_SPLIT, :]
                )
                prefetch(ck + 3)
                nc.sync.dma_start(
                    xout_r[:, :, ck, OUT_SPLIT:], o_sb[:, :, OUT_SPLIT:, :]
                )
    nc.compile()
    return nc


def _get_nc():
    if "nc" not in _CACHE:
        _CACHE["nc"] = _build()
    return _CACHE["nc"]


def kernel(x_a, x_v, x_d, W_ttrans, W_ttrace, W_tdelta, W_vd, W_dv, **_ignored):
    x_a = np.asarray(x_a, dtype=np.float32)
    x_v = np.asarray(x_v, dtype=np.float32)
    x_d = np.asarray(x_d, dtype=np.float32)
    W_ttrans = np.asarray(W_ttrans, dtype=np.float32)
    W_ttrace = np.asarray(W_ttrace, dtype=np.float32)
    W_tdelta = np.asarray(W_tdelta, dtype=np.float32)
    W_vd = np.asarray(W_vd, dtype=np.float32)
    W_dv = np.asarray(W_dv, dtype=np.float32)

    eye = np.eye(C, dtype=np.float32)
    W_ct = (W_ttrace @ (eye + W_ttrans)).astype(np.float32)
    W_cd = (W_dv @ (eye - W_ttrans)).astype(np.float32)
    wts = np.ascontiguousarray(
        np.stack([
            W_ttrans.T, W_tdelta.T, W_vd.T, -W_vd.T, W_cd.T, W_ct.T,
        ]).astype(np.float32)
    )

    # host reshard: pack (x_d, x_v, x_a) chunk-plane-major, channel on outer dim
    xin = np.empty((NCORES, C, NCHUNK, NPL, TCH), dtype=np.float32)
    xin[:, :, :, 0:9, :] = (
        x_d.reshape(NCORES, NCHUNK, TCH, C, 9).transpose(0, 3, 1, 4, 2)
    )
    xin[:, :, :, 9:12, :] = (
        x_v.reshape(NCORES, NCHUNK, TCH, C, 3).transpose(0, 3, 1, 4, 2)
    )
    xin[:, :, :, 12, :] = (
        x_a.reshape(NCORES, NCHUNK, TCH, C).transpose(0, 3, 1, 2)
    )

    in_maps = [{"xin": xin[c], "wts": wts} for c in range(NCORES)]

    nc = _get_nc()
    res = run_bass_kernel_spmd(nc, in_maps, core_ids=list(range(NCORES)))

    xout = np.stack([res.results[c]["xout"] for c in range(NCORES)])
    # device returns fp16 deltas; residual add happens here in exact fp32
    # [core, C, chunk, plane, t] -> [core, chunk, t, C, plane]
    delta = xout.transpose(0, 2, 4, 1, 3).astype(np.float32)
    x_d_out = x_d + np.ascontiguousarray(delta[..., 0:9]).reshape(B, N, C, 3, 3)
    x_v_out = x_v + np.ascontiguousarray(delta[..., 9:12]).reshape(B, N, C, 3)
    x_a_out = x_a + np.ascontiguousarray(delta[..., 12]).reshape(B, N, C)
    return (x_a_out, x_v_out, x_d_out)


# revision 21
# speedup vs baseline: 1.4723x; 1.1290x over previous
"""Trainium2 Bass kernel for nn_AVDFullLinearMix.

Math (folded form, all terms single matmuls over raw inputs):
  x_d_out = x_d + W_ttrans @ x_d^T(spatial) + (W_tdelta @ x_a) * eye3
            + eps_expand(W_vd @ x_v)
  x_a_out = x_a + W_ct @ trace(x_d),        W_ct = W_ttrace @ (I + W_ttrans)
  x_v_out = x_v + W_cd @ eps_contract(x_d), W_cd = W_dv @ (I - W_ttrans)
(the TensDelta term never reaches the eps-contract since eps[i,i,k] = 0,
 and trace/eps-contract commute with the TensTrans spatial transpose up to
 identity/negation, so both weight chains fold on the host.)

Sharding: data-parallel over 8 cores (2048 tokens each); weights replicated.
Device layout is channel-major and chunk-packed: one input tensor
[256, chunk, 13, tok] per core holding (9 x_d planes, 3 x_v planes, 1 x_a)
so each chunk is ONE contiguous in-DMA (sync ring) while outputs stream on
the scalar HWDGE ring. Matmuls run as float32r (fp22 multiply, fp32
accumulate) at full PE rate; both output-channel halves share one PSUM bank
so each spatial plane needs a single DVE combine op.
"""

import numpy as np

import concourse.bass as bass  # noqa: F401  (registers handle types)
import concourse.mybir as mybir
import concourse.tile as tile
from concourse import bacc
from concourse.bass_utils import run_bass_kernel_spmd

NCORES = 8
P = 128          # partitions
C = 256          # channels
B, N = 16, 1024
T = (B * N) // NCORES   # tokens per core = 2048
TCH = 256               # tokens per chunk
NCHUNK = T // TCH       # 8
NPL = 13                # planes per chunk: 9 x_d, 3 x_v, 1 x_a
PL_XV = 9
PL_XA = 12
OUT_SPLIT = 7           # out-DMA part A = planes [0:7), part B = [7:13)

F32 = mybir.dt.float32
F16 = mybir.dt.float16
R32 = mybir.dt.float32r

# flat spatial index s = i*3 + j
SPERM = [0, 3, 6, 1, 4, 7, 2, 5, 8]    # s -> transposed flat (j*3+i)
DIAG = (0, 4, 8)
# off-diag s=(k,j): x_d_out[...,k,j] += eps[i*,j,k] * (W_vd @ x_v[..,i*])
# VD_MAP: s -> (i*, weight slot) with slot 2 = +W_vd^T, 3 = -W_vd^T
VD_MAP = {1: (2, 3), 2: (1, 2), 3: (2, 2), 5: (0, 3), 6: (1, 3), 7: (0, 2)}
# eps-contract y[:,k] = x_d[:,s1] - x_d[:,s2]
Y_PAIRS = [(5, 7), (6, 2), (1, 3)]
# weight slots (lhsT layout W.T): 0=ttrans 1=tdelta 2=+vd 3=-vd 4=cd 5=ct
NW = 6

_CACHE: dict = {}


def _build():
    nc = bacc.Bacc(None, target_bir_lowering=False)
    xin = nc.dram_tensor("xin", [C, NCHUNK, NPL, TCH], F16, kind="ExternalInput")
    wts = nc.dram_tensor("wts", [NW, C, C], F16, kind="ExternalInput")
    xout = nc.dram_tensor("xout", [C, NCHUNK, NPL, TCH], F16, kind="ExternalOutput")

    # channel-split dram views: c = h*128 + p
    xin_r = xin.rearrange("(h p) n s t -> p h n s t", p=P)
    xout_r = xout.rearrange("(h p) n s t -> p h n s t", p=P)
    wts_r = wts.rearrange("w (kh p) o -> p w kh o", p=P)

    with tile.TileContext(nc) as tc:
        with (
            tc.tile_pool(name="wpool", bufs=1) as wpool,
            tc.tile_pool(name="data", bufs=3) as data,
            tc.tile_pool(name="outp", bufs=3) as outp,
            tc.tile_pool(name="tmp", bufs=2) as tmp,
            tc.tile_pool(name="psum", bufs=8, space="PSUM") as psum,
        ):
            w_sb = wpool.tile([P, NW, 2, C], F16)
            nc.sync.dma_start(w_sb[:], wts_r)

            def lhsT(w, kh, mh):
                return w_sb[:, w, kh, mh * P:(mh + 1) * P]

            x_tiles = {}

            def prefetch(ck, part=None):
                if ck >= NCHUNK:
                    return
                if ck not in x_tiles:
                    x_tiles[ck] = data.tile([P, 2, NPL, TCH], F16, name="x", tag="x")
                xt = x_tiles[ck]
                lo, hi = {
                    None: (0, NPL), 0: (0, OUT_SPLIT), 1: (OUT_SPLIT, NPL)
                }[part]
                nc.sync.dma_start(
                    xt[:, :, lo:hi, :], xin_r[:, :, ck, lo:hi]
                )

            # 3-chunk in-DMA lookahead keeps the sync HWDGE ring free of
            # head-of-line blocking; in/out parts alternate so reads and
            # writes overlap in the SDMA engines (higher aggregate HBM BW)
            for ck in range(3):
                prefetch(ck)

            for ck in range(NCHUNK):
                x_sb = x_tiles[ck]
                o_sb = outp.tile([P, 2, NPL, TCH], F16, tag="o")

                # eps-contract and trace of raw x_d; fp32r-typed for the matmuls
                y_sb = tmp.tile([P, 2, 3, TCH], F16, tag="y")
                tr_sb = tmp.tile([P, 2, TCH], F16, tag="tr")
                for k, (s1, s2) in enumerate(Y_PAIRS):
                    nc.vector.tensor_sub(
                        y_sb[:, :, k, :], x_sb[:, :, s1, :], x_sb[:, :, s2, :]
                    )
                nc.vector.tensor_add(tr_sb[:], x_sb[:, :, 0, :], x_sb[:, :, 4, :])
                nc.vector.tensor_add(tr_sb[:], tr_sb[:], x_sb[:, :, 8, :])

                def mm_group(ps, terms):
                    """ps[:, mh, :] (+)= sum of W[wslot].T @ rhs(kh) terms, both
                    output-channel halves into one PSUM bank."""
                    for mh in range(2):
                        n = 0
                        total = len(terms) * 2
                        for wslot, rhs in terms:
                            for kh in range(2):
                                n += 1
                                nc.tensor.matmul(
                                    ps[:, mh, :], lhsT(wslot, kh, mh), rhs(kh),
                                    start=(n == 1), stop=(n == total),
                                )

                # x_d planes
                for s in range(9):
                    ps = psum.tile([P, 2, TCH], F32, tag="ps")
                    if s in DIAG:
                        extra = (1, lambda kh: x_sb[:, kh, PL_XA, :])
                    else:
                        istar, wslot = VD_MAP[s]
                        extra = (
                            wslot,
                            lambda kh, i=istar: x_sb[:, kh, PL_XV + i, :],
                        )
                    mm_group(ps, [
                        (0, lambda kh, sp=SPERM[s]: x_sb[:, kh, sp, :]),
                        extra,
                    ])
                    nc.vector.tensor_copy(o_sb[:, :, s, :], ps[:])

                # x_v planes: xv += W_cd @ y
                for k in range(3):
                    ps = psum.tile([P, 2, TCH], F32, tag="ps")
                    mm_group(ps, [
                        (4, lambda kh, kk=k: y_sb[:, kh, kk, :]),
                    ])
                    nc.vector.tensor_copy(o_sb[:, :, PL_XV + k, :], ps[:])

                # x_a plane: xa += W_ct @ tr
                ps = psum.tile([P, 2, TCH], F32, tag="ps")
                mm_group(ps, [
                    (5, lambda kh: tr_sb[:, kh, :]),
                ])
                nc.vector.tensor_copy(o_sb[:, :, PL_XA, :], ps[:])

                nc.sync.dma_start(
                    xout_r[:, :, ck, :OUT_SPLIT], o_sb[:, :, :OUT_SPLIT, :]
                )
                prefetch(ck + 3)
                nc.sync.dma_start(
                    xout_r[:, :, ck, OUT_SPLIT:], o_sb[:, :, OUT_SPLIT:, :]
                )
    nc.compile()
    return nc


def _get_nc():
    if "nc" not in _CACHE:
        _CACHE["nc"] = _build()
    return _CACHE["nc"]


def kernel(x_a, x_v, x_d, W_ttrans, W_ttrace, W_tdelta, W_vd, W_dv, **_ignored):
    x_a = np.asarray(x_a, dtype=np.float32)
    x_v = np.asarray(x_v, dtype=np.float32)
    x_d = np.asarray(x_d, dtype=np.float32)
    W_ttrans = np.asarray(W_ttrans, dtype=np.float32)
    W_ttrace = np.asarray(W_ttrace, dtype=np.float32)
    W_tdelta = np.asarray(W_tdelta, dtype=np.float32)
    W_vd = np.asarray(W_vd, dtype=np.float32)
    W_dv = np.asarray(W_dv, dtype=np.float32)

    eye = np.eye(C, dtype=np.float32)
    W_ct = (W_ttrace @ (eye + W_ttrans)).astype(np.float32)
    W_cd = (W_dv @ (eye - W_ttrans)).astype(np.float32)
    wts = np.ascontiguousarray(
        np.stack([
            W_ttrans.T, W_tdelta.T, W_vd.T, -W_vd.T, W_cd.T, W_ct.T,
        ]).astype(np.float16)
    )

    # host reshard: pack (x_d, x_v, x_a) chunk-plane-major, channel on outer dim
    xin = np.empty((NCORES, C, NCHUNK, NPL, TCH), dtype=np.float16)
    xin[:, :, :, 0:9, :] = (
        x_d.reshape(NCORES, NCHUNK, TCH, C, 9).transpose(0, 3, 1, 4, 2)
    )
    xin[:, :, :, 9:12, :] = (
        x_v.reshape(NCORES, NCHUNK, TCH, C, 3).transpose(0, 3, 1, 4, 2)
    )
    xin[:, :, :, 12, :] = (
        x_a.reshape(NCORES, NCHUNK, TCH, C).transpose(0, 3, 1, 2)
    )

    in_maps = [{"xin": xin[c], "wts": wts} for c in range(NCORES)]

    nc = _get_nc()
    res = run_bass_kernel_spmd(nc, in_maps, core_ids=list(range(NCORES)))

    xout = np.stack([res.results[c]["xout"] for c in range(NCORES)])
    # device returns fp16 deltas; residual add happens here in exact fp32
    # [core, C, chunk, plane, t] -> [core, chunk, t, C, plane]
    delta = xout.transpose(0, 2, 4, 1, 3).astype(np.float32)
    x_d_out = x_d + np.ascontiguousarray(delta[..., 0:9]).reshape(B, N, C, 3, 3)
    x_v_out = x_v + np.ascontiguousarray(delta[..., 9:12]).reshape(B, N, C, 3)
    x_a_out = x_a + np.ascontiguousarray(delta[..., 12]).reshape(B, N, C)
    return (x_a_out, x_v_out, x_d_out)


# revision 22
# speedup vs baseline: 1.5005x; 1.0192x over previous
"""Trainium2 Bass kernel for nn_AVDFullLinearMix.

Math (folded form, all terms single matmuls over raw inputs):
  x_d_out = x_d + W_ttrans @ x_d^T(spatial) + (W_tdelta @ x_a) * eye3
            + eps_expand(W_vd @ x_v)
  x_a_out = x_a + W_ct @ trace(x_d),        W_ct = W_ttrace @ (I + W_ttrans)
  x_v_out = x_v + W_cd @ eps_contract(x_d), W_cd = W_dv @ (I - W_ttrans)
(the TensDelta term never reaches the eps-contract since eps[i,i,k] = 0,
 and trace/eps-contract commute with the TensTrans spatial transpose up to
 identity/negation, so both weight chains fold on the host.)

The device computes and returns only the DELTAS in fp16; the exact-fp32
residual add (x + delta) happens on the host. This halves both stream
directions (inputs also stream fp16 since every consumer goes through the
fp16/fp22 matmul datapath anyway).

Sharding: data-parallel over 8 cores (2048 tokens each); weights replicated.
Device layout is channel-major and chunk-packed: one input tensor
[256, chunk, 13, tok] per core holding (9 x_d planes, 3 x_v planes, 1 x_a).
The sync HWDGE ring carries everything with a 3-chunk in-DMA lookahead so
the FIFO never head-of-line blocks, and in/out transfers alternate so HBM
reads and writes overlap. The shared delta (W_tdelta @ x_a) and the three
vd products (W_vd @ x_v) are computed once per chunk and reused across the
diagonal / off-diagonal planes via the DVE combine (add/sub handles the
Levi-Civita sign).
"""

import numpy as np

import concourse.bass as bass  # noqa: F401  (registers handle types)
import concourse.mybir as mybir
import concourse.tile as tile
from concourse import bacc
from concourse.bass_utils import run_bass_kernel_spmd

NCORES = 8
P = 128          # partitions
C = 256          # channels
B, N = 16, 1024
T = (B * N) // NCORES   # tokens per core = 2048
TCH = 512               # tokens per chunk
NCHUNK = T // TCH       # 4
NPL = 13                # planes per chunk: 9 x_d, 3 x_v, 1 x_a
PL_XV = 9
PL_XA = 12
OUT_SPLIT = 7           # out-DMA part A = planes [0:7), part B = [7:13)

F32 = mybir.dt.float32
F16 = mybir.dt.float16

# flat spatial index s = i*3 + j
SPERM = [0, 3, 6, 1, 4, 7, 2, 5, 8]    # s -> transposed flat (j*3+i)
DIAG = (0, 4, 8)
# off-diag s=(k,j): x_d_out[...,k,j] += eps[i*,j,k] * (W_vd @ x_v[..,i*])
# VD_MAP: s -> (i*, sign)
VD_MAP = {1: (2, -1), 2: (1, +1), 3: (2, +1), 5: (0, -1), 6: (1, -1), 7: (0, +1)}
# eps-contract y[:,k] = x_d[:,s1] - x_d[:,s2]
Y_PAIRS = [(5, 7), (6, 2), (1, 3)]
# weight slots (lhsT layout W.T): 0=ttrans 1=tdelta 2=vd 3=cd 4=ct
NW = 5

_CACHE: dict = {}


def _build():
    nc = bacc.Bacc(None, target_bir_lowering=False)
    xin = nc.dram_tensor("xin", [C, NCHUNK, NPL, TCH], F16, kind="ExternalInput")
    wts = nc.dram_tensor("wts", [NW, C, C], F16, kind="ExternalInput")
    xout = nc.dram_tensor("xout", [C, NCHUNK, NPL, TCH], F16, kind="ExternalOutput")

    # channel-split dram views: c = h*128 + p
    xin_r = xin.rearrange("(h p) n s t -> p h n s t", p=P)
    xout_r = xout.rearrange("(h p) n s t -> p h n s t", p=P)
    wts_r = wts.rearrange("w (kh p) o -> p w kh o", p=P)

    with tile.TileContext(nc) as tc:
        with (
            tc.tile_pool(name="wpool", bufs=1) as wpool,
            tc.tile_pool(name="data", bufs=3) as data,
            tc.tile_pool(name="outp", bufs=3) as outp,
            tc.tile_pool(name="tmp", bufs=2) as tmp,
            tc.tile_pool(name="psum", bufs=4, space="PSUM") as psum,
        ):
            w_sb = wpool.tile([P, NW, 2, C], F16)
            nc.sync.dma_start(w_sb[:], wts_r)

            def lhsT(w, kh, mh):
                return w_sb[:, w, kh, mh * P:(mh + 1) * P]

            x_tiles = {}

            def prefetch(ck):
                if ck >= NCHUNK:
                    return
                xt = data.tile([P, 2, NPL, TCH], F16, name="x", tag="x")
                x_tiles[ck] = xt
                nc.sync.dma_start(xt[:], xin_r[:, :, ck])

            # 3-chunk in-DMA lookahead keeps the sync HWDGE ring free of
            # head-of-line blocking; in/out transfers alternate so HBM reads
            # and writes overlap in the SDMA engines
            for ck in range(3):
                prefetch(ck)

            for ck in range(NCHUNK):
                x_sb = x_tiles.pop(ck)
                o_sb = outp.tile([P, 2, NPL, TCH], F16, tag="o")

                # eps-contract and trace of raw x_d
                y_sb = tmp.tile([P, 2, 3, TCH], F16, tag="y")
                tr_sb = tmp.tile([P, 2, TCH], F16, tag="tr")
                for k, (s1, s2) in enumerate(Y_PAIRS):
                    nc.vector.tensor_sub(
                        y_sb[:, :, k, :], x_sb[:, :, s1, :], x_sb[:, :, s2, :]
                    )
                nc.vector.tensor_add(tr_sb[:], x_sb[:, :, 0, :], x_sb[:, :, 4, :])
                nc.vector.tensor_add(tr_sb[:], tr_sb[:], x_sb[:, :, 8, :])

                def mm_group(ps, wslot, rhs):
                    """ps[:, mh, :] = W[wslot].T @ rhs, both output-channel
                    halves into one two-bank PSUM tile."""
                    for mh in range(2):
                        for kh in range(2):
                            nc.tensor.matmul(
                                ps[:, mh, :], lhsT(wslot, kh, mh), rhs(kh),
                                start=(kh == 0), stop=(kh == 1),
                            )

                # shared products, computed once per chunk
                dl_sb = tmp.tile([P, 2, TCH], F16, tag="dl")
                ps = psum.tile([P, 2, TCH], F32, tag="ps")
                mm_group(ps, 1, lambda kh: x_sb[:, kh, PL_XA, :])
                nc.vector.tensor_copy(dl_sb[:], ps[:])

                vd_sb = tmp.tile([P, 2, 3, TCH], F16, tag="vd")
                for i in range(3):
                    ps = psum.tile([P, 2, TCH], F32, tag="ps")
                    mm_group(ps, 2, lambda kh, i=i: x_sb[:, kh, PL_XV + i, :])
                    nc.vector.tensor_copy(vd_sb[:, :, i, :], ps[:])

                # x_d planes: delta_d = ttrans +/- (delta | vd)
                for s in range(9):
                    ps = psum.tile([P, 2, TCH], F32, tag="ps")
                    mm_group(ps, 0, lambda kh, sp=SPERM[s]: x_sb[:, kh, sp, :])
                    if s in DIAG:
                        nc.vector.tensor_add(o_sb[:, :, s, :], ps[:], dl_sb[:])
                    else:
                        istar, sign = VD_MAP[s]
                        op = nc.vector.tensor_add if sign > 0 else nc.vector.tensor_sub
                        op(o_sb[:, :, s, :], ps[:], vd_sb[:, :, istar, :])

                # x_v planes: delta_v = W_cd @ y
                for k in range(3):
                    ps = psum.tile([P, 2, TCH], F32, tag="ps")
                    mm_group(ps, 3, lambda kh, kk=k: y_sb[:, kh, kk, :])
                    nc.vector.tensor_copy(o_sb[:, :, PL_XV + k, :], ps[:])

                # x_a plane: delta_a = W_ct @ tr
                ps = psum.tile([P, 2, TCH], F32, tag="ps")
                mm_group(ps, 4, lambda kh: tr_sb[:, kh, :])
                nc.vector.tensor_copy(o_sb[:, :, PL_XA, :], ps[:])

                nc.sync.dma_start(
                    xout_r[:, :, ck, :OUT_SPLIT], o_sb[:, :, :OUT_SPLIT, :]
                )
                prefetch(ck + 3)
                nc.sync.dma_start(
                    xout_r[:, :, ck, OUT_SPLIT:], o_sb[:, :, OUT_SPLIT:, :]
                )
    nc.compile()
    return nc


def _get_nc():
    if "nc" not in _CACHE:
        _CACHE["nc"] = _build()
    return _CACHE["nc"]


def kernel(x_a, x_v, x_d, W_ttrans, W_ttrace, W_tdelta, W_vd, W_dv, **_ignored):
    x_a = np.asarray(x_a, dtype=np.float32)
    x_v = np.asarray(x_v, dtype=np.float32)
    x_d = np.asarray(x_d, dtype=np.float32)
    W_ttrans = np.asarray(W_ttrans, dtype=np.float32)
    W_ttrace = np.asarray(W_ttrace, dtype=np.float32)
    W_tdelta = np.asarray(W_tdelta, dtype=np.float32)
    W_vd = np.asarray(W_vd, dtype=np.float32)
    W_dv = np.asarray(W_dv, dtype=np.float32)

    eye = np.eye(C, dtype=np.float32)
    W_ct = (W_ttrace @ (eye + W_ttrans)).astype(np.float32)
    W_cd = (W_dv @ (eye - W_ttrans)).astype(np.float32)
    wts = np.ascontiguousarray(
        np.stack([W_ttrans.T, W_tdelta.T, W_vd.T, W_cd.T, W_ct.T]).astype(np.float16)
    )

    # host reshard: pack (x_d, x_v, x_a) chunk-plane-major, channel outer, fp16
    xin = np.empty((NCORES, C, NCHUNK, NPL, TCH), dtype=np.float16)
    xin[:, :, :, 0:9, :] = (
        x_d.reshape(NCORES, NCHUNK, TCH, C, 9).transpose(0, 3, 1, 4, 2)
    )
    xin[:, :, :, 9:12, :] = (
        x_v.reshape(NCORES, NCHUNK, TCH, C, 3).transpose(0, 3, 1, 4, 2)
    )
    xin[:, :, :, 12, :] = (
        x_a.reshape(NCORES, NCHUNK, TCH, C).transpose(0, 3, 1, 2)
    )

    in_maps = [{"xin": xin[c], "wts": wts} for c in range(NCORES)]

    nc = _get_nc()
    res = run_bass_kernel_spmd(nc, in_maps, core_ids=list(range(NCORES)))

    xout = np.stack([res.results[c]["xout"] for c in range(NCORES)])
    # device returns fp16 deltas; residual add happens here in exact fp32
    # [core, C, chunk, plane, t] -> [core, chunk, t, C, plane]
    delta = xout.transpose(0, 2, 4, 1, 3).astype(np.float32)
    x_d_out = x_d + np.ascontiguousarray(delta[..., 0:9]).reshape(B, N, C, 3, 3)
    x_v_out = x_v + np.ascontiguousarray(delta[..., 9:12]).reshape(B, N, C, 3)
    x_a_out = x_a + np.ascontiguousarray(delta[..., 12]).reshape(B, N, C)
    return (x_a_out, x_v_out, x_d_out)
